# revision 34
# baseline (speedup 1.0000x reference)
"""Distributed Trainium2 Bass kernel for multi-head attention.

Problem: B=4, S=2048, D=1024, 16 heads (depth 64), f32, mask all-ones.

Sharding (8 cores): data-parallel over batch (4) x tensor-parallel over
heads (2 groups of 8 heads). Core c handles batch c//2, head-group c%2.
Each core computes a partial out-projection (its 8 heads' contribution);
the host sums the two partials per batch and adds the bias.

Per-core pipeline (all matmuls bf16 into f32 PSUM):
  - inputs arrive pre-transposed/pre-sliced from host: xT [1024,2048],
    wq/wk/wv [1024,512], wo [512,1024], all bf16.
  - KT/QT computed in transposed layout [d_head on partitions, seq free]
    via lhsT=w chunk, rhs=xT chunk.
  - V computed in natural [keys, hd] layout via lhsT=xT chunk, rhs=wv,
    stored per (key-tile, head) with an extra all-ones column (ones-trick:
    the attn@V matmul then also produces the softmax denominator).
  - logits^T tiles [128 keys, 512 q] on PSUM; exp via ScalarE activation
    with scale=1/8 folded in (no max-subtraction needed: logits are O(1)).
  - attn@V: lhsT = V[keys,65], rhs = exp tile -> psum [65, 512 q]
    accumulated over key tiles; row 64 = denominator.
  - normalize: DVE reciprocal of denominator row, broadcast across 64
    partitions via a DRAM-bounce DMA, multiply.  Odd heads additionally
    bounce through an SBUF->SBUF DMA to land on partitions 64:128
    (compute engines cannot shift partitions).
  - out-proj: lhsT = attn_outT [hd chunk, q tile], rhs = wo chunk,
    accumulated over 4 hd chunks -> partial y [q, 1024] f32, DMA'd out.
"""

import os
import sys

for _p in ("/opt/trn_rl_repo", "/opt/pypackages"):
    if _p not in sys.path and os.path.isdir(_p):
        sys.path.append(_p)

import ml_dtypes
import numpy as np

import concourse.tile as tile
from concourse import bacc, mybir
from concourse.bass_utils import run_bass_kernel_spmd

P = 128
SEQ = 2048
DM = 1024          # model dim
HDIM = 512         # heads*depth per core (8 heads x 64)
NH = 8             # heads per core
DH = 64            # head depth
KK = DM // P       # 8 contraction chunks of d_model
HC = HDIM // P     # 4 hd chunks (head pairs)
QCW = 512          # q-chunk width
GW = 2             # key-tiles per exp group

F32 = mybir.dt.float32
BF16 = mybir.dt.bfloat16
AF = mybir.ActivationFunctionType

_NC_CACHE = {}


def build(seq=SEQ, interleave=True, fast_recip=True):
    nst = seq // P       # key tiles
    nqc = seq // QCW     # q chunks
    nqt = QCW // P       # q tiles per chunk

    nc = bacc.Bacc(
        "TRN2",
        target_bir_lowering=False,
        debug=False,
        enable_asserts=True,
        num_devices=8,
    )
    xT_d = nc.dram_tensor("xT", [DM, seq], BF16, kind="ExternalInput").ap()
    wq_d = nc.dram_tensor("wq", [DM, HDIM], BF16, kind="ExternalInput").ap()
    wk_d = nc.dram_tensor("wk", [DM, HDIM], BF16, kind="ExternalInput").ap()
    wv_d = nc.dram_tensor("wv", [DM, HDIM], BF16, kind="ExternalInput").ap()
    wo_d = nc.dram_tensor("wo", [HDIM, DM], BF16, kind="ExternalInput").ap()
    out_d = nc.dram_tensor("out", [seq, DM], F32, kind="ExternalOutput").ap()

    with tile.TileContext(nc) as tc:
        with (
            tc.tile_pool(name="persist", bufs=1) as persist,
            tc.tile_pool(name="wpool", bufs=1) as wpool,
            # bpool: 4 shared [128,512] psum banks for attention-out (po)
            # and QKV/proj accumulators; spsum: 2x 2-bank logits groups.
            tc.tile_pool(name="bpool", bufs=4, space="PSUM") as bpool,
            tc.tile_pool(name="spsum", bufs=2, space="PSUM") as spsum,
            tc.tile_pool(name="ptp", bufs=6) as ptp,
            tc.tile_pool(name="rp", bufs=6) as rp,
            tc.tile_pool(name="rbcp", bufs=6) as rbcp,
            tc.tile_pool(name="tnp", bufs=3) as tnp,
            tc.tile_pool(name="ysbp", bufs=4) as ysbp,
            tc.tile_pool(name="dramp", bufs=8, space="DRAM") as dramp,
        ):
            ppsum = bpool
            QT = persist.tile([P, HC, seq], BF16)
            KT = persist.tile([P, HC, seq], BF16)
            V = persist.tile([P, nst, NH, DH + 1], BF16)
            AO = persist.tile([P, HC, seq], BF16)
            wo = persist.tile([P, HC, DM], BF16)
            # per-chunk tiles so region deps release per-DMA (compute ramps
            # with the loads instead of waiting for the full tensor)
            xT = [persist.tile([P, seq], BF16, name=f"xT{kk}") for kk in range(KK)]
            wq = [wpool.tile([P, HDIM], BF16, name=f"wq{kk}") for kk in range(KK)]
            wk = [wpool.tile([P, HDIM], BF16, name=f"wk{kk}") for kk in range(KK)]
            wv = [wpool.tile([P, HDIM], BF16, name=f"wv{kk}") for kk in range(KK)]

            # input DMAs: wk/xT chunk pairs first (KT production consumes
            # them in kk order), alternating two queue engines for bandwidth
            # three queues for aggregate bandwidth, but the first wk/xT pair
            # rides the two hardware-DGE queues (sync, scalar) -- gpsimd is
            # software-DGE with slow first-byte descriptor generation
            _dengines = [nc.sync, nc.scalar, nc.gpsimd]

            def deng(i):
                return _dengines[i % 3]

            for kk in range(KK):
                deng(kk).dma_start(wk[kk][:], wk_d[kk * P : (kk + 1) * P, :])
                deng(kk + 1).dma_start(xT[kk][:], xT_d[kk * P : (kk + 1) * P, :])
            for kk in range(KK):
                deng(kk).dma_start(wv[kk][:], wv_d[kk * P : (kk + 1) * P, :])
            for kk in range(KK):
                deng(kk).dma_start(wq[kk][:], wq_d[kk * P : (kk + 1) * P, :])
            for c in range(HC):
                deng(c).dma_start(wo[:, c, :], wo_d[c * P : (c + 1) * P, :])
            # ones column for the denominator trick: preset whole V to 1,
            # value regions get overwritten by the V copies below.
            nc.any.memset(V[:], 1.0)

            def kt_block(m, kb):
                ps = ppsum.tile([P, QCW], F32, tag="b512", name=f"ktps_{m}_{kb}")
                for kk in range(KK):
                    nc.tensor.matmul(
                        ps[:],
                        wk[kk][:, m * P : (m + 1) * P],
                        xT[kk][:, kb * QCW : (kb + 1) * QCW],
                        start=(kk == 0),
                        stop=(kk == KK - 1),
                    )
                nc.vector.tensor_copy(KT[:, m, kb * QCW : (kb + 1) * QCW], ps[:])

            def v_half(st, half):
                ps = ppsum.tile([P, QCW // 2], F32, tag="b512", name=f"vps_{st}_{half}")
                for kk in range(KK):
                    nc.tensor.matmul(
                        ps[:],
                        xT[kk][:, st * P : (st + 1) * P],
                        wv[kk][:, half * (QCW // 2) : (half + 1) * (QCW // 2)],
                        start=(kk == 0),
                        stop=(kk == KK - 1),
                    )
                nc.vector.tensor_copy(
                    V[:, st, half * (NH // 2) : (half + 1) * (NH // 2), 0:DH],
                    ps[:].rearrange("p (h d) -> p h d", h=NH // 2),
                )

            def qt_block(qcc, m):
                qss = slice(qcc * QCW, (qcc + 1) * QCW)
                ps = ppsum.tile([P, QCW], F32, tag="b512")
                for kk in range(KK):
                    nc.tensor.matmul(
                        ps[:],
                        wq[kk][:, m * P : (m + 1) * P],
                        xT[kk][:, qss],
                        start=(kk == 0),
                        stop=(kk == KK - 1),
                    )
                nc.vector.tensor_copy(QT[:, m, qss], ps[:])

            def proj_tile(qcc, slot):
                qt, oc = slot // 2, slot % 2
                row0 = qcc * QCW + qt * P
                ps = ppsum.tile([P, QCW], F32, tag="b512")
                for c in range(HC):
                    nc.tensor.matmul(
                        ps[:],
                        AO[:, c, row0 : row0 + P],
                        wo[:, c, oc * QCW : (oc + 1) * QCW],
                        start=(c == 0),
                        stop=(c == HC - 1),
                    )
                ys = ysbp.tile([P, QCW], F32, tag="ys")
                nc.vector.tensor_copy(ys[:], ps[:])
                nc.sync.dma_start(
                    out_d[row0 : row0 + P, oc * QCW : (oc + 1) * QCW], ys[:]
                )

            def qt_steps(qcc, m):
                """qt_block split into 8 single-matmul filler steps."""
                state = {}
                qss = slice(qcc * QCW, (qcc + 1) * QCW)

                def step(kk):
                    if kk == 0:
                        state["ps"] = ppsum.tile(
                            [P, QCW], F32, tag="b512", name=f"qsps_{qcc}_{m}"
                        )
                    nc.tensor.matmul(
                        state["ps"][:],
                        wq[kk][:, m * P : (m + 1) * P],
                        xT[kk][:, qss],
                        start=(kk == 0),
                        stop=(kk == KK - 1),
                    )
                    if kk == KK - 1:
                        nc.vector.tensor_copy(QT[:, m, qss], state["ps"][:])

                return [lambda kk=kk: step(kk) for kk in range(KK)]

            def proj_steps(qcc, slot):
                """proj_tile split into 4 single-matmul filler steps."""
                state = {}
                qt, oc = slot // 2, slot % 2
                row0 = qcc * QCW + qt * P

                def step(c):
                    if c == 0:
                        state["ps"] = ppsum.tile(
                            [P, QCW], F32, tag="b512", name=f"prps_{qcc}_{slot}"
                        )
                    nc.tensor.matmul(
                        state["ps"][:],
                        AO[:, c, row0 : row0 + P],
                        wo[:, c, oc * QCW : (oc + 1) * QCW],
                        start=(c == 0),
                        stop=(c == HC - 1),
                    )
                    if c == HC - 1:
                        ys = ysbp.tile([P, QCW], F32, tag="ys")
                        nc.vector.tensor_copy(ys[:], state["ps"][:])
                        nc.sync.dma_start(
                            out_d[row0 : row0 + P, oc * QCW : (oc + 1) * QCW], ys[:]
                        )

                return [lambda c=c: step(c) for c in range(HC)]

            # ---- minimal prologue: only what attention pair 0, groups 0-1
            # need ----
            for kb in range(nqc):
                kt_block(0, kb)
            qt_block(0, 0)
            for st in range(4):
                v_half(st, 0)

            # Filler queues, one list per q chunk. Each entry emits a small
            # amount of TensorE work; entries are popped between S^T(u+1)
            # and attn@V(u) so the PE always has an independent matmul in
            # flight while the attn@V waits on ScalarE's exp semaphore.
            # qc0 carries the remaining KT/QT/V production (deadlines in
            # comments: unit index by which the result is consumed).
            fillers = {qc: [] for qc in range(nqc)}
            f0 = fillers[0]
            for st in range(4, nst):          # deadline: unit st//2 (pair 0)
                f0.append(lambda st=st: v_half(st, 0))
            for kb in range(nqc):             # deadline: unit 16 (pair 1)
                f0.append(lambda kb=kb: kt_block(1, kb))
            f0.append(lambda: qt_block(0, 1))
            for kb in range(nqc):             # deadline: unit 32 (pair 2)
                f0.append(lambda kb=kb: kt_block(2, kb))
            f0.append(lambda: qt_block(0, 2))
            for st in range(0, NH):           # V half 1 (heads 4-7): unit 32+
                f0.append(lambda st=st: v_half(st, 1))
            for kb in range(nqc):             # deadline: unit 48 (pair 3)
                f0.append(lambda kb=kb: kt_block(3, kb))
            f0.append(lambda: qt_block(0, 3))
            for st in range(NH, nst):         # deadline: unit 32 + st//2
                f0.append(lambda st=st: v_half(st, 1))
            if nqc > 1:
                f0.append(lambda: qt_block(1, 0))
            for qc in range(1, nqc):
                fl = fillers[qc]
                if qc == 1:
                    # QT(1, m>=1) moved out of PE-bound qc0 into qc1's slack;
                    # needed by qc1's pair m at unit 16m (2 pops/unit early)
                    for m in range(1, HC):
                        fl.extend(qt_steps(1, m))
                qts = (
                    [qt_steps(qc + 1, m) for m in range(HC)] if qc + 1 < nqc else []
                )
                prs = [proj_steps(qc - 1, s) for s in range(NH)]
                blocks = []
                for i in range(HC):
                    if i < len(qts):
                        blocks.append(qts[i])
                    blocks.append(prs[2 * i])
                    blocks.append(prs[2 * i + 1])
                for b in blocks:
                    fl.extend(b)

            def normalize(po, h, qc):
                """attn-out = po[0:64] * (1 / po[64]) -> AO[head slot]."""
                m, off = h // 2, (h % 2) * DH
                qs = slice(qc * QCW, (qc + 1) * QCW)
                rt = rp.tile([DH + 1, QCW], F32, tag="rt")
                nc.vector.tensor_copy(rt[DH : DH + 1, :], po[DH : DH + 1, :])
                rd = dramp.tile([1, QCW], F32, tag="rd")
                nc.sync.dma_start(rd[:], rt[DH : DH + 1, :])
                dbc = rp.tile([DH, QCW], F32, tag="dbc")
                nc.sync.dma_start(dbc[:], rd[0:1, :].to_broadcast((DH, QCW)))
                rbc = rbcp.tile([DH, QCW], F32, tag="rbc")
                if fast_recip:
                    nc.vector.reciprocal_approx_fast(rbc[:], dbc[:])
                else:
                    nc.vector.reciprocal(rbc[:], dbc[:])
                if off == 0:
                    nc.vector.tensor_mul(AO[0:DH, m, qs], po[0:DH, :], rbc[:])
                else:
                    tn = tnp.tile([DH, QCW], BF16, tag="tn")
                    nc.vector.tensor_mul(tn[:], po[0:DH, :], rbc[:])
                    # partition shift 0:64 -> 64:128 (engines can't)
                    nc.sync.dma_start(AO[DH:P, m, qs], tn[:])

            ngrp = nst // GW

            def st_group(h, qc, g):
                """logits^T matmuls for key-tile group g of head h."""
                m, off = h // 2, (h % 2) * DH
                qs = slice(qc * QCW, (qc + 1) * QCW)
                sg = spsum.tile([P, GW, QCW], F32, tag="sg")
                for j in range(GW):
                    st = g * GW + j
                    nc.tensor.matmul(
                        sg[:, j, :],
                        KT[off : off + DH, m, st * P : (st + 1) * P],
                        QT[off : off + DH, m, qs],
                        start=True,
                        stop=True,
                    )
                return sg

            # Flattened, 1-deep software-pipelined attention stream: the PE
            # order is S^T(u+1) BEFORE attn@V(u), so the logits of the next
            # group are ready the moment ScalarE finishes exp(u) -- ScalarE
            # (the attention-phase bottleneck) never starves.
            sg_next = st_group(0, 0, 0)
            for qc in range(nqc):
                units = [(h, g) for h in range(NH) for g in range(ngrp)]
                fl = fillers[qc] if interleave else []
                po = {}
                for idx, (h, g) in enumerate(units):
                    sg = sg_next
                    pt = ptp.tile([P, GW, QCW], BF16, tag="pt")
                    nc.scalar.activation(pt[:], sg[:], AF.Exp, scale=0.125)
                    if idx + 1 < len(units):
                        hn, gn = units[idx + 1]
                        sg_next = st_group(hn, qc, gn)
                    elif qc + 1 < nqc:
                        sg_next = st_group(0, qc + 1, 0)
                    if g == 0:
                        po[h] = bpool.tile(
                            [P, QCW], F32, tag="b512", name=f"po_{qc}_{h}"
                        )
                    # filler work BEFORE attn@V: the independent matmul(s)
                    # execute while attn@V waits on the exp semaphore
                    if qc == 0:
                        npop = 2 if idx < 6 else 1
                    elif qc == 1:
                        npop = 2 if idx < 24 else 1
                    else:
                        npop = 1
                    for _ in range(npop):
                        if fl:
                            fl.pop(0)()
                    for j in range(GW):
                        st = g * GW + j
                        nc.tensor.matmul(
                            po[h][0 : DH + 1, :],
                            V[:, st, h, :],
                            pt[:, j, :],
                            start=(st == 0),
                            stop=(st == nst - 1),
                            skip_group_check=True,
                        )
                    if g == ngrp - 1:
                        normalize(po.pop(h), h, qc)

                if not interleave:
                    if qc + 1 < nqc:
                        for m in range(HC):
                            qt_block(qc + 1, m)
                    for slot in range(NH):
                        proj_tile(qc, slot)

            if interleave:
                # epilogue: out-proj of the last q chunk
                for slot in range(NH):
                    proj_tile(nqc - 1, slot)

    nc.compile()
    return nc


def get_nc(seq=SEQ):
    if seq not in _NC_CACHE:
        _NC_CACHE[seq] = build(seq)
    return _NC_CACHE[seq]


def make_in_maps(x, wq, wk, wv, wo):
    bf = ml_dtypes.bfloat16
    in_maps = []
    for c in range(8):
        b, g = c // 2, c % 2
        gs = slice(g * HDIM, (g + 1) * HDIM)
        in_maps.append(
            {
                "xT": np.ascontiguousarray(np.asarray(x)[b].T).astype(bf),
                "wq": np.ascontiguousarray(np.asarray(wq)[:, gs]).astype(bf),
                "wk": np.ascontiguousarray(np.asarray(wk)[:, gs]).astype(bf),
                "wv": np.ascontiguousarray(np.asarray(wv)[:, gs]).astype(bf),
                "wo": np.ascontiguousarray(np.asarray(wo)[gs, :]).astype(bf),
            }
        )
    return in_maps


def combine_outputs(results, bo):
    outs = [np.asarray(results[c]["out"], dtype=np.float32) for c in range(8)]
    y = np.stack([outs[2 * b] + outs[2 * b + 1] for b in range(4)])
    return (y + np.asarray(bo, dtype=np.float32).reshape(1, 1, -1)).astype(np.float32)


def kernel(x, mask, wq, wk, wv, wo, bo):
    nc = get_nc()
    in_maps = make_in_maps(x, wq, wk, wv, wo)
    res = run_bass_kernel_spmd(nc, in_maps, core_ids=list(range(8)))
    return combine_outputs(res.results, bo)


# revision 35
# speedup vs baseline: 1.0068x; 1.0068x over previous
"""Distributed Trainium2 Bass kernel for multi-head attention.

Problem: B=4, S=2048, D=1024, 16 heads (depth 64), f32, mask all-ones.

Sharding (8 cores): data-parallel over batch (4) x tensor-parallel over
heads (2 groups of 8 heads). Core c handles batch c//2, head-group c%2.
Each core computes a partial out-projection (its 8 heads' contribution);
the host sums the two partials per batch and adds the bias.

Per-core pipeline (all matmuls bf16 into f32 PSUM):
  - inputs arrive pre-transposed/pre-sliced from host: xT [1024,2048],
    wq/wk/wv [1024,512], wo [512,1024], all bf16.
  - KT/QT computed in transposed layout [d_head on partitions, seq free]
    via lhsT=w chunk, rhs=xT chunk.
  - V computed in natural [keys, hd] layout via lhsT=xT chunk, rhs=wv,
    stored per (key-tile, head) with an extra all-ones column (ones-trick:
    the attn@V matmul then also produces the softmax denominator).
  - logits^T tiles [128 keys, 512 q] on PSUM; exp via ScalarE activation
    with scale=1/8 folded in (no max-subtraction needed: logits are O(1)).
  - attn@V: lhsT = V[keys,65], rhs = exp tile -> psum [65, 512 q]
    accumulated over key tiles; row 64 = denominator.
  - normalize: DVE reciprocal of denominator row, broadcast across 64
    partitions via a DRAM-bounce DMA, multiply.  Odd heads additionally
    bounce through an SBUF->SBUF DMA to land on partitions 64:128
    (compute engines cannot shift partitions).
  - out-proj: lhsT = attn_outT [hd chunk, q tile], rhs = wo chunk,
    accumulated over 4 hd chunks -> partial y [q, 1024] f32, DMA'd out.
"""

import os
import sys

for _p in ("/opt/trn_rl_repo", "/opt/pypackages"):
    if _p not in sys.path and os.path.isdir(_p):
        sys.path.append(_p)

import ml_dtypes
import numpy as np

import concourse.tile as tile
from concourse import bacc, mybir
from concourse.bass_utils import run_bass_kernel_spmd

P = 128
SEQ = 2048
DM = 1024          # model dim
HDIM = 512         # heads*depth per core (8 heads x 64)
NH = 8             # heads per core
DH = 64            # head depth
KK = DM // P       # 8 contraction chunks of d_model
HC = HDIM // P     # 4 hd chunks (head pairs)
QCW = 512          # q-chunk width
GW = 2             # key-tiles per exp group

F32 = mybir.dt.float32
BF16 = mybir.dt.bfloat16
AF = mybir.ActivationFunctionType

_NC_CACHE = {}


def build(seq=SEQ, interleave=True, fast_recip=True):
    nst = seq // P       # key tiles
    nqc = seq // QCW     # q chunks
    nqt = QCW // P       # q tiles per chunk

    nc = bacc.Bacc(
        "TRN2",
        target_bir_lowering=False,
        debug=False,
        enable_asserts=True,
        num_devices=8,
    )
    xT_d = nc.dram_tensor("xT", [DM, seq], BF16, kind="ExternalInput").ap()
    wq_d = nc.dram_tensor("wq", [DM, HDIM], BF16, kind="ExternalInput").ap()
    wk_d = nc.dram_tensor("wk", [DM, HDIM], BF16, kind="ExternalInput").ap()
    wv_d = nc.dram_tensor("wv", [DM, HDIM], BF16, kind="ExternalInput").ap()
    wo_d = nc.dram_tensor("wo", [HDIM, DM], BF16, kind="ExternalInput").ap()
    out_d = nc.dram_tensor("out", [seq, DM], F32, kind="ExternalOutput").ap()

    with tile.TileContext(nc) as tc:
        with (
            tc.tile_pool(name="persist", bufs=1) as persist,
            tc.tile_pool(name="wpool", bufs=1) as wpool,
            # bpool: 4 shared [128,512] psum banks for attention-out (po)
            # and QKV/proj accumulators; spsum: 2x 2-bank logits groups.
            tc.tile_pool(name="bpool", bufs=4, space="PSUM") as bpool,
            tc.tile_pool(name="spsum", bufs=2, space="PSUM") as spsum,
            tc.tile_pool(name="ptp", bufs=6) as ptp,
            tc.tile_pool(name="rp", bufs=6) as rp,
            tc.tile_pool(name="rbcp", bufs=6) as rbcp,
            tc.tile_pool(name="tnp", bufs=3) as tnp,
            tc.tile_pool(name="ysbp", bufs=4) as ysbp,
            tc.tile_pool(name="dramp", bufs=8, space="DRAM") as dramp,
        ):
            ppsum = bpool
            QT = persist.tile([P, HC, seq], BF16)
            KT = persist.tile([P, HC, seq], BF16)
            V = persist.tile([P, nst, NH, DH + 1], BF16)
            AO = persist.tile([P, HC, seq], BF16)
            wo = persist.tile([P, HC, DM], BF16)
            # per-chunk tiles so region deps release per-DMA (compute ramps
            # with the loads instead of waiting for the full tensor)
            xT = [persist.tile([P, seq], BF16, name=f"xT{kk}") for kk in range(KK)]
            wq = [wpool.tile([P, HDIM], BF16, name=f"wq{kk}") for kk in range(KK)]
            wk = [wpool.tile([P, HDIM], BF16, name=f"wk{kk}") for kk in range(KK)]
            wv = [wpool.tile([P, HDIM], BF16, name=f"wv{kk}") for kk in range(KK)]

            # input DMAs: wk/xT chunk pairs first (KT production consumes
            # them in kk order), alternating two queue engines for bandwidth
            _dengines = [nc.sync, nc.gpsimd, nc.scalar]

            def deng(i):
                return _dengines[i % 3]

            for kk in range(KK):
                deng(kk).dma_start(wk[kk][:], wk_d[kk * P : (kk + 1) * P, :])
                deng(kk + 1).dma_start(xT[kk][:], xT_d[kk * P : (kk + 1) * P, :])
            for kk in range(KK):
                deng(kk).dma_start(wv[kk][:], wv_d[kk * P : (kk + 1) * P, :])
            for kk in range(KK):
                deng(kk).dma_start(wq[kk][:], wq_d[kk * P : (kk + 1) * P, :])
            for c in range(HC):
                deng(c).dma_start(wo[:, c, :], wo_d[c * P : (c + 1) * P, :])
            # ones column for the denominator trick: preset whole V to 1,
            # value regions get overwritten by the V copies below.
            nc.any.memset(V[:], 1.0)

            def kt_block(m, kb):
                ps = ppsum.tile([P, QCW], F32, tag="b512", name=f"ktps_{m}_{kb}")
                for kk in range(KK):
                    nc.tensor.matmul(
                        ps[:],
                        wk[kk][:, m * P : (m + 1) * P],
                        xT[kk][:, kb * QCW : (kb + 1) * QCW],
                        start=(kk == 0),
                        stop=(kk == KK - 1),
                    )
                nc.vector.tensor_copy(KT[:, m, kb * QCW : (kb + 1) * QCW], ps[:])

            def v_half(st, half):
                ps = ppsum.tile([P, QCW // 2], F32, tag="b512", name=f"vps_{st}_{half}")
                for kk in range(KK):
                    nc.tensor.matmul(
                        ps[:],
                        xT[kk][:, st * P : (st + 1) * P],
                        wv[kk][:, half * (QCW // 2) : (half + 1) * (QCW // 2)],
                        start=(kk == 0),
                        stop=(kk == KK - 1),
                    )
                nc.vector.tensor_copy(
                    V[:, st, half * (NH // 2) : (half + 1) * (NH // 2), 0:DH],
                    ps[:].rearrange("p (h d) -> p h d", h=NH // 2),
                )

            def qt_block(qcc, m):
                qss = slice(qcc * QCW, (qcc + 1) * QCW)
                ps = ppsum.tile([P, QCW], F32, tag="b512")
                for kk in range(KK):
                    nc.tensor.matmul(
                        ps[:],
                        wq[kk][:, m * P : (m + 1) * P],
                        xT[kk][:, qss],
                        start=(kk == 0),
                        stop=(kk == KK - 1),
                    )
                nc.vector.tensor_copy(QT[:, m, qss], ps[:])

            def proj_tile(qcc, slot):
                qt, oc = slot // 2, slot % 2
                row0 = qcc * QCW + qt * P
                ps = ppsum.tile([P, QCW], F32, tag="b512")
                for c in range(HC):
                    nc.tensor.matmul(
                        ps[:],
                        AO[:, c, row0 : row0 + P],
                        wo[:, c, oc * QCW : (oc + 1) * QCW],
                        start=(c == 0),
                        stop=(c == HC - 1),
                    )
                ys = ysbp.tile([P, QCW], F32, tag="ys")
                nc.vector.tensor_copy(ys[:], ps[:])
                nc.sync.dma_start(
                    out_d[row0 : row0 + P, oc * QCW : (oc + 1) * QCW], ys[:]
                )

            def qt_steps(qcc, m):
                """qt_block split into 8 single-matmul filler steps."""
                state = {}
                qss = slice(qcc * QCW, (qcc + 1) * QCW)

                def step(kk):
                    if kk == 0:
                        state["ps"] = ppsum.tile(
                            [P, QCW], F32, tag="b512", name=f"qsps_{qcc}_{m}"
                        )
                    nc.tensor.matmul(
                        state["ps"][:],
                        wq[kk][:, m * P : (m + 1) * P],
                        xT[kk][:, qss],
                        start=(kk == 0),
                        stop=(kk == KK - 1),
                    )
                    if kk == KK - 1:
                        nc.vector.tensor_copy(QT[:, m, qss], state["ps"][:])

                return [lambda kk=kk: step(kk) for kk in range(KK)]

            def proj_steps(qcc, slot):
                """proj_tile split into 4 single-matmul filler steps."""
                state = {}
                qt, oc = slot // 2, slot % 2
                row0 = qcc * QCW + qt * P

                def step(c):
                    if c == 0:
                        state["ps"] = ppsum.tile(
                            [P, QCW], F32, tag="b512", name=f"prps_{qcc}_{slot}"
                        )
                    nc.tensor.matmul(
                        state["ps"][:],
                        AO[:, c, row0 : row0 + P],
                        wo[:, c, oc * QCW : (oc + 1) * QCW],
                        start=(c == 0),
                        stop=(c == HC - 1),
                    )
                    if c == HC - 1:
                        ys = ysbp.tile([P, QCW], F32, tag="ys")
                        nc.vector.tensor_copy(ys[:], state["ps"][:])
                        nc.sync.dma_start(
                            out_d[row0 : row0 + P, oc * QCW : (oc + 1) * QCW], ys[:]
                        )

                return [lambda c=c: step(c) for c in range(HC)]

            # ---- minimal prologue: only what attention pair 0, groups 0-1
            # need ----
            for kb in range(nqc):
                kt_block(0, kb)
            qt_block(0, 0)
            for st in range(4):
                v_half(st, 0)

            # Filler queues, one list per q chunk. Each entry emits a small
            # amount of TensorE work; entries are popped between S^T(u+1)
            # and attn@V(u) so the PE always has an independent matmul in
            # flight while the attn@V waits on ScalarE's exp semaphore.
            # qc0 carries the remaining KT/QT/V production (deadlines in
            # comments: unit index by which the result is consumed).
            fillers = {qc: [] for qc in range(nqc)}
            f0 = fillers[0]
            for st in range(4, nst):          # deadline: unit st//2 (pair 0)
                f0.append(lambda st=st: v_half(st, 0))
            for kb in range(nqc):             # deadline: unit 16 (pair 1)
                f0.append(lambda kb=kb: kt_block(1, kb))
            f0.append(lambda: qt_block(0, 1))
            for kb in range(nqc):             # deadline: unit 32 (pair 2)
                f0.append(lambda kb=kb: kt_block(2, kb))
            f0.append(lambda: qt_block(0, 2))
            for st in range(0, NH):           # V half 1 (heads 4-7): unit 32+
                f0.append(lambda st=st: v_half(st, 1))
            for kb in range(nqc):             # deadline: unit 48 (pair 3)
                f0.append(lambda kb=kb: kt_block(3, kb))
            f0.append(lambda: qt_block(0, 3))
            for st in range(NH, nst):         # deadline: unit 32 + st//2
                f0.append(lambda st=st: v_half(st, 1))
            if nqc > 1:
                f0.append(lambda: qt_block(1, 0))
            for qc in range(1, nqc):
                fl = fillers[qc]
                if qc == 1:
                    # QT(1, m>=1) moved out of PE-bound qc0 into qc1's slack;
                    # needed by qc1's pair m at unit 16m (2 pops/unit early)
                    for m in range(1, HC):
                        fl.extend(qt_steps(1, m))
                qts = (
                    [qt_steps(qc + 1, m) for m in range(HC)] if qc + 1 < nqc else []
                )
                prs = [proj_steps(qc - 1, s) for s in range(NH)]
                blocks = []
                for i in range(HC):
                    if i < len(qts):
                        blocks.append(qts[i])
                    blocks.append(prs[2 * i])
                    blocks.append(prs[2 * i + 1])
                for b in blocks:
                    fl.extend(b)

            def normalize(po, h, qc):
                """attn-out = po[0:64] * (1 / po[64]) -> AO[head slot]."""
                m, off = h // 2, (h % 2) * DH
                qs = slice(qc * QCW, (qc + 1) * QCW)
                rt = rp.tile([DH + 1, QCW], F32, tag="rt")
                nc.vector.tensor_copy(rt[DH : DH + 1, :], po[DH : DH + 1, :])
                rd = dramp.tile([1, QCW], F32, tag="rd")
                nc.sync.dma_start(rd[:], rt[DH : DH + 1, :])
                dbc = rp.tile([DH, QCW], F32, tag="dbc")
                nc.sync.dma_start(dbc[:], rd[0:1, :].to_broadcast((DH, QCW)))
                rbc = rbcp.tile([DH, QCW], F32, tag="rbc")
                if fast_recip:
                    nc.vector.reciprocal_approx_fast(rbc[:], dbc[:])
                else:
                    nc.vector.reciprocal(rbc[:], dbc[:])
                if off == 0:
                    nc.vector.tensor_mul(AO[0:DH, m, qs], po[0:DH, :], rbc[:])
                else:
                    tn = tnp.tile([DH, QCW], BF16, tag="tn")
                    nc.vector.tensor_mul(tn[:], po[0:DH, :], rbc[:])
                    # partition shift 0:64 -> 64:128 (engines can't)
                    nc.sync.dma_start(AO[DH:P, m, qs], tn[:])

            ngrp = nst // GW

            def st_group(h, qc, g):
                """logits^T matmuls for key-tile group g of head h."""
                m, off = h // 2, (h % 2) * DH
                qs = slice(qc * QCW, (qc + 1) * QCW)
                sg = spsum.tile([P, GW, QCW], F32, tag="sg")
                for j in range(GW):
                    st = g * GW + j
                    nc.tensor.matmul(
                        sg[:, j, :],
                        KT[off : off + DH, m, st * P : (st + 1) * P],
                        QT[off : off + DH, m, qs],
                        start=True,
                        stop=True,
                    )
                return sg

            # Flattened, 1-deep software-pipelined attention stream: the PE
            # order is S^T(u+1) BEFORE attn@V(u), so the logits of the next
            # group are ready the moment ScalarE finishes exp(u) -- ScalarE
            # (the attention-phase bottleneck) never starves.
            sg_next = st_group(0, 0, 0)
            for qc in range(nqc):
                units = [(h, g) for h in range(NH) for g in range(ngrp)]
                fl = fillers[qc] if interleave else []
                po = {}
                for idx, (h, g) in enumerate(units):
                    sg = sg_next
                    pt = ptp.tile([P, GW, QCW], BF16, tag="pt")
                    nc.scalar.activation(pt[:], sg[:], AF.Exp, scale=0.125)
                    if idx + 1 < len(units):
                        hn, gn = units[idx + 1]
                        sg_next = st_group(hn, qc, gn)
                    elif qc + 1 < nqc:
                        sg_next = st_group(0, qc + 1, 0)
                    if g == 0:
                        po[h] = bpool.tile(
                            [P, QCW], F32, tag="b512", name=f"po_{qc}_{h}"
                        )
                    # filler work BEFORE attn@V: the independent matmul(s)
                    # execute while attn@V waits on the exp semaphore
                    if qc == 0:
                        npop = 2 if idx < 6 else 1
                    elif qc == 1:
                        npop = 2 if idx < 24 else 1
                    else:
                        npop = 1
                    for _ in range(npop):
                        if fl:
                            fl.pop(0)()
                    for j in range(GW):
                        st = g * GW + j
                        nc.tensor.matmul(
                            po[h][0 : DH + 1, :],
                            V[:, st, h, :],
                            pt[:, j, :],
                            start=(st == 0),
                            stop=(st == nst - 1),
                            skip_group_check=True,
                        )
                    if g == ngrp - 1:
                        normalize(po.pop(h), h, qc)

                if not interleave:
                    if qc + 1 < nqc:
                        for m in range(HC):
                            qt_block(qc + 1, m)
                    for slot in range(NH):
                        proj_tile(qc, slot)

            if interleave:
                # epilogue: out-proj of the last q chunk
                for slot in range(NH):
                    proj_tile(nqc - 1, slot)

    nc.compile()
    return nc


def get_nc(seq=SEQ):
    if seq not in _NC_CACHE:
        _NC_CACHE[seq] = build(seq)
    return _NC_CACHE[seq]


def make_in_maps(x, wq, wk, wv, wo):
    bf = ml_dtypes.bfloat16
    in_maps = []
    for c in range(8):
        b, g = c // 2, c % 2
        gs = slice(g * HDIM, (g + 1) * HDIM)
        in_maps.append(
            {
                "xT": np.ascontiguousarray(np.asarray(x)[b].T).astype(bf),
                "wq": np.ascontiguousarray(np.asarray(wq)[:, gs]).astype(bf),
                "wk": np.ascontiguousarray(np.asarray(wk)[:, gs]).astype(bf),
                "wv": np.ascontiguousarray(np.asarray(wv)[:, gs]).astype(bf),
                "wo": np.ascontiguousarray(np.asarray(wo)[gs, :]).astype(bf),
            }
        )
    return in_maps


def combine_outputs(results, bo):
    outs = [np.asarray(results[c]["out"], dtype=np.float32) for c in range(8)]
    y = np.stack([outs[2 * b] + outs[2 * b + 1] for b in range(4)])
    return (y + np.asarray(bo, dtype=np.float32).reshape(1, 1, -1)).astype(np.float32)


def kernel(x, mask, wq, wk, wv, wo, bo):
    nc = get_nc()
    in_maps = make_in_maps(x, wq, wk, wv, wo)
    res = run_bass_kernel_spmd(nc, in_maps, core_ids=list(range(8)))
    return combine_outputs(res.results, bo)


# revision 37
# speedup vs baseline: 1.0069x; 1.0001x over previous
"""Distributed Trainium2 Bass kernel for multi-head attention.

Problem: B=4, S=2048, D=1024, 16 heads (depth 64), f32, mask all-ones.

Sharding (8 cores): data-parallel over batch (4) x tensor-parallel over
heads (2 groups of 8 heads). Core c handles batch c//2, head-group c%2.
Each core computes a partial out-projection (its 8 heads' contribution);
the host sums the two partials per batch and adds the bias.

Per-core pipeline (all matmuls bf16 into f32 PSUM):
  - inputs arrive pre-transposed/pre-sliced from host: xT [1024,2048],
    wq/wk/wv [1024,512], wo [512,1024], all bf16.
  - KT/QT computed in transposed layout [d_head on partitions, seq free]
    via lhsT=w chunk, rhs=xT chunk.
  - V computed in natural [keys, hd] layout via lhsT=xT chunk, rhs=wv,
    stored per (key-tile, head) with an extra all-ones column (ones-trick:
    the attn@V matmul then also produces the softmax denominator).
  - logits^T tiles [128 keys, 512 q] on PSUM; exp via ScalarE activation
    with scale=1/8 folded in (no max-subtraction needed: logits are O(1)).
  - attn@V: lhsT = V[keys,65], rhs = exp tile -> psum [65, 512 q]
    accumulated over key tiles; row 64 = denominator.
  - normalize: DVE reciprocal of denominator row, broadcast across 64
    partitions via a DRAM-bounce DMA, multiply.  Odd heads additionally
    bounce through an SBUF->SBUF DMA to land on partitions 64:128
    (compute engines cannot shift partitions).
  - out-proj: lhsT = attn_outT [hd chunk, q tile], rhs = wo chunk,
    accumulated over 4 hd chunks -> partial y [q, 1024] f32, DMA'd out.
"""

import os
import sys

for _p in ("/opt/trn_rl_repo", "/opt/pypackages"):
    if _p not in sys.path and os.path.isdir(_p):
        sys.path.append(_p)

import ml_dtypes
import numpy as np

import concourse.tile as tile
from concourse import bacc, mybir
from concourse.bass_utils import run_bass_kernel_spmd

P = 128
SEQ = 2048
DM = 1024          # model dim
HDIM = 512         # heads*depth per core (8 heads x 64)
NH = 8             # heads per core
DH = 64            # head depth
KK = DM // P       # 8 contraction chunks of d_model
HC = HDIM // P     # 4 hd chunks (head pairs)
QCW = 512          # q-chunk width
GW = 2             # key-tiles per exp group

F32 = mybir.dt.float32
BF16 = mybir.dt.bfloat16
AF = mybir.ActivationFunctionType

_NC_CACHE = {}


def build(seq=SEQ, interleave=True, fast_recip=True):
    nst = seq // P       # key tiles
    nqc = seq // QCW     # q chunks
    nqt = QCW // P       # q tiles per chunk

    nc = bacc.Bacc(
        "TRN2",
        target_bir_lowering=False,
        debug=False,
        enable_asserts=True,
        num_devices=8,
    )
    xT_d = nc.dram_tensor("xT", [DM, seq], BF16, kind="ExternalInput").ap()
    wq_d = nc.dram_tensor("wq", [DM, HDIM], BF16, kind="ExternalInput").ap()
    wk_d = nc.dram_tensor("wk", [DM, HDIM], BF16, kind="ExternalInput").ap()
    wv_d = nc.dram_tensor("wv", [DM, HDIM], BF16, kind="ExternalInput").ap()
    wo_d = nc.dram_tensor("wo", [HDIM, DM], BF16, kind="ExternalInput").ap()
    out_d = nc.dram_tensor("out", [seq, DM], F32, kind="ExternalOutput").ap()

    with tile.TileContext(nc) as tc:
        with (
            tc.tile_pool(name="persist", bufs=1) as persist,
            tc.tile_pool(name="wpool", bufs=1) as wpool,
            # bpool: 4 shared [128,512] psum banks for attention-out (po)
            # and QKV/proj accumulators; spsum: 2x 2-bank logits groups.
            tc.tile_pool(name="bpool", bufs=4, space="PSUM") as bpool,
            tc.tile_pool(name="spsum", bufs=2, space="PSUM") as spsum,
            tc.tile_pool(name="ptp", bufs=6) as ptp,
            tc.tile_pool(name="rp", bufs=6) as rp,
            tc.tile_pool(name="rbcp", bufs=6) as rbcp,
            tc.tile_pool(name="tnp", bufs=3) as tnp,
            tc.tile_pool(name="ysbp", bufs=4) as ysbp,
            tc.tile_pool(name="dramp", bufs=8, space="DRAM") as dramp,
        ):
            ppsum = bpool
            QT = persist.tile([P, HC, seq], BF16)
            KT = persist.tile([P, HC, seq], BF16)
            V = persist.tile([P, nst, NH, DH + 1], BF16)
            AO = persist.tile([P, HC, seq], BF16)
            wo = persist.tile([P, HC, DM], BF16)
            # per-chunk tiles so region deps release per-DMA (compute ramps
            # with the loads instead of waiting for the full tensor)
            xT = [persist.tile([P, seq], BF16, name=f"xT{kk}") for kk in range(KK)]
            wq = [wpool.tile([P, HDIM], BF16, name=f"wq{kk}") for kk in range(KK)]
            wk = [wpool.tile([P, HDIM], BF16, name=f"wk{kk}") for kk in range(KK)]
            wv = [wpool.tile([P, HDIM], BF16, name=f"wv{kk}") for kk in range(KK)]

            # input DMAs: wk/xT chunk pairs first (KT production consumes
            # them in kk order), alternating two queue engines for bandwidth
            _dengines = [nc.sync, nc.gpsimd, nc.scalar]

            def deng(i):
                return _dengines[i % 3]

            for kk in range(KK):
                deng(kk).dma_start(wk[kk][:], wk_d[kk * P : (kk + 1) * P, :])
                deng(kk + 1).dma_start(xT[kk][:], xT_d[kk * P : (kk + 1) * P, :])
            for kk in range(KK):
                deng(kk).dma_start(wv[kk][:], wv_d[kk * P : (kk + 1) * P, :])
            for kk in range(KK):
                deng(kk).dma_start(wq[kk][:], wq_d[kk * P : (kk + 1) * P, :])
            for c in range(HC):
                deng(c).dma_start(wo[:, c, :], wo_d[c * P : (c + 1) * P, :])
            # ones column for the denominator trick: preset whole V to 1,
            # value regions get overwritten by the V copies below.
            nc.any.memset(V[:], 1.0)

            def kt_block(m, kb):
                ps = ppsum.tile([P, QCW], F32, tag="b512", name=f"ktps_{m}_{kb}")
                for kk in range(KK):
                    nc.tensor.matmul(
                        ps[:],
                        wk[kk][:, m * P : (m + 1) * P],
                        xT[kk][:, kb * QCW : (kb + 1) * QCW],
                        start=(kk == 0),
                        stop=(kk == KK - 1),
                    )
                nc.vector.tensor_copy(KT[:, m, kb * QCW : (kb + 1) * QCW], ps[:])

            def v_half(st, half):
                ps = ppsum.tile([P, QCW // 2], F32, tag="b512", name=f"vps_{st}_{half}")
                for kk in range(KK):
                    nc.tensor.matmul(
                        ps[:],
                        xT[kk][:, st * P : (st + 1) * P],
                        wv[kk][:, half * (QCW // 2) : (half + 1) * (QCW // 2)],
                        start=(kk == 0),
                        stop=(kk == KK - 1),
                    )
                nc.vector.tensor_copy(
                    V[:, st, half * (NH // 2) : (half + 1) * (NH // 2), 0:DH],
                    ps[:].rearrange("p (h d) -> p h d", h=NH // 2),
                )

            def qt_block(qcc, m):
                qss = slice(qcc * QCW, (qcc + 1) * QCW)
                ps = ppsum.tile([P, QCW], F32, tag="b512")
                for kk in range(KK):
                    nc.tensor.matmul(
                        ps[:],
                        wq[kk][:, m * P : (m + 1) * P],
                        xT[kk][:, qss],
                        start=(kk == 0),
                        stop=(kk == KK - 1),
                    )
                nc.vector.tensor_copy(QT[:, m, qss], ps[:])

            def proj_tile(qcc, slot):
                qt, oc = slot // 2, slot % 2
                row0 = qcc * QCW + qt * P
                ps = ppsum.tile([P, QCW], F32, tag="b512")
                for c in range(HC):
                    nc.tensor.matmul(
                        ps[:],
                        AO[:, c, row0 : row0 + P],
                        wo[:, c, oc * QCW : (oc + 1) * QCW],
                        start=(c == 0),
                        stop=(c == HC - 1),
                    )
                ys = ysbp.tile([P, QCW], F32, tag="ys")
                nc.vector.tensor_copy(ys[:], ps[:])
                nc.sync.dma_start(
                    out_d[row0 : row0 + P, oc * QCW : (oc + 1) * QCW], ys[:]
                )

            def qt_steps(qcc, m):
                """qt_block split into 8 single-matmul filler steps."""
                state = {}
                qss = slice(qcc * QCW, (qcc + 1) * QCW)

                def step(kk):
                    if kk == 0:
                        state["ps"] = ppsum.tile(
                            [P, QCW], F32, tag="b512", name=f"qsps_{qcc}_{m}"
                        )
                    nc.tensor.matmul(
                        state["ps"][:],
                        wq[kk][:, m * P : (m + 1) * P],
                        xT[kk][:, qss],
                        start=(kk == 0),
                        stop=(kk == KK - 1),
                    )
                    if kk == KK - 1:
                        nc.vector.tensor_copy(QT[:, m, qss], state["ps"][:])

                return [lambda kk=kk: step(kk) for kk in range(KK)]

            def proj_steps(qcc, slot):
                """proj_tile split into 4 single-matmul filler steps."""
                state = {}
                qt, oc = slot // 2, slot % 2
                row0 = qcc * QCW + qt * P

                def step(c):
                    if c == 0:
                        state["ps"] = ppsum.tile(
                            [P, QCW], F32, tag="b512", name=f"prps_{qcc}_{slot}"
                        )
                    nc.tensor.matmul(
                        state["ps"][:],
                        AO[:, c, row0 : row0 + P],
                        wo[:, c, oc * QCW : (oc + 1) * QCW],
                        start=(c == 0),
                        stop=(c == HC - 1),
                    )
                    if c == HC - 1:
                        ys = ysbp.tile([P, QCW], F32, tag="ys")
                        nc.vector.tensor_copy(ys[:], state["ps"][:])
                        nc.sync.dma_start(
                            out_d[row0 : row0 + P, oc * QCW : (oc + 1) * QCW], ys[:]
                        )

                return [lambda c=c: step(c) for c in range(HC)]

            # ---- minimal prologue: only what attention pair 0, groups 0-1
            # need ----
            for kb in range(nqc):
                kt_block(0, kb)
            qt_block(0, 0)
            for st in range(4):
                v_half(st, 0)

            # Filler queues, one list per q chunk. Each entry emits a small
            # amount of TensorE work; entries are popped between S^T(u+1)
            # and attn@V(u) so the PE always has an independent matmul in
            # flight while the attn@V waits on ScalarE's exp semaphore.
            # qc0 carries the remaining KT/QT/V production (deadlines in
            # comments: unit index by which the result is consumed).
            fillers = {qc: [] for qc in range(nqc)}
            f0 = fillers[0]
            for st in range(4, nst):          # deadline: unit st//2 (pair 0)
                f0.append(lambda st=st: v_half(st, 0))
            for kb in range(nqc):             # deadline: unit 16 (pair 1)
                f0.append(lambda kb=kb: kt_block(1, kb))
            f0.append(lambda: qt_block(0, 1))
            for kb in range(nqc):             # deadline: unit 32 (pair 2)
                f0.append(lambda kb=kb: kt_block(2, kb))
            f0.append(lambda: qt_block(0, 2))
            for st in range(0, NH):           # V half 1 (heads 4-7): unit 32+
                f0.append(lambda st=st: v_half(st, 1))
            for kb in range(nqc):             # deadline: unit 48 (pair 3)
                f0.append(lambda kb=kb: kt_block(3, kb))
            f0.append(lambda: qt_block(0, 3))
            for st in range(NH, nst):         # deadline: unit 32 + st//2
                f0.append(lambda st=st: v_half(st, 1))
            if nqc > 1:
                f0.append(lambda: qt_block(1, 0))
            for qc in range(1, nqc):
                fl = fillers[qc]
                if qc == 1:
                    # QT(1, m>=1) moved out of PE-bound qc0 into qc1's slack;
                    # needed by qc1's pair m at unit 16m (2 pops/unit early)
                    for m in range(1, HC):
                        fl.extend(qt_steps(1, m))
                qts = (
                    [qt_steps(qc + 1, m) for m in range(HC)] if qc + 1 < nqc else []
                )
                prs = [proj_steps(qc - 1, s) for s in range(NH)]
                blocks = []
                for i in range(HC):
                    if i < len(qts):
                        blocks.append(qts[i])
                    blocks.append(prs[2 * i])
                    blocks.append(prs[2 * i + 1])
                for b in blocks:
                    fl.extend(b)

            def normalize(po, h, qc):
                """attn-out = po[0:64] * (1 / po[64]) -> AO[head slot]."""
                m, off = h // 2, (h % 2) * DH
                qs = slice(qc * QCW, (qc + 1) * QCW)
                rt = rp.tile([DH + 1, QCW], F32, tag="rt")
                nc.vector.tensor_copy(rt[DH : DH + 1, :], po[DH : DH + 1, :])
                rd = dramp.tile([1, QCW], F32, tag="rd")
                nc.sync.dma_start(rd[:], rt[DH : DH + 1, :])
                dbc = rp.tile([DH, QCW], F32, tag="dbc")
                nc.sync.dma_start(dbc[:], rd[0:1, :].to_broadcast((DH, QCW)))
                rbc = rbcp.tile([DH, QCW], F32, tag="rbc")
                if fast_recip:
                    nc.vector.reciprocal_approx_fast(rbc[:], dbc[:])
                else:
                    nc.vector.reciprocal(rbc[:], dbc[:])
                if off == 0:
                    nc.vector.tensor_mul(AO[0:DH, m, qs], po[0:DH, :], rbc[:])
                else:
                    tn = tnp.tile([DH, QCW], BF16, tag="tn")
                    nc.vector.tensor_mul(tn[:], po[0:DH, :], rbc[:])
                    # partition shift 0:64 -> 64:128 (engines can't)
                    nc.sync.dma_start(AO[DH:P, m, qs], tn[:])

            ngrp = nst // GW

            def st_group(h, qc, g):
                """logits^T matmuls for key-tile group g of head h."""
                m, off = h // 2, (h % 2) * DH
                qs = slice(qc * QCW, (qc + 1) * QCW)
                sg = spsum.tile([P, GW, QCW], F32, tag="sg")
                for j in range(GW):
                    st = g * GW + j
                    nc.tensor.matmul(
                        sg[:, j, :],
                        KT[off : off + DH, m, st * P : (st + 1) * P],
                        QT[off : off + DH, m, qs],
                        start=True,
                        stop=True,
                    )
                return sg

            # Flattened, 1-deep software-pipelined attention stream: the PE
            # order is S^T(u+1) BEFORE attn@V(u), so the logits of the next
            # group are ready the moment ScalarE finishes exp(u) -- ScalarE
            # (the attention-phase bottleneck) never starves.
            sg_next = st_group(0, 0, 0)
            for qc in range(nqc):
                units = [(h, g) for h in range(NH) for g in range(ngrp)]
                fl = fillers[qc] if interleave else []
                po = {}
                for idx, (h, g) in enumerate(units):
                    sg = sg_next
                    pt = ptp.tile([P, GW, QCW], BF16, tag="pt")
                    nc.scalar.activation(pt[:], sg[:], AF.Exp, scale=0.125)
                    if idx + 1 < len(units):
                        hn, gn = units[idx + 1]
                        sg_next = st_group(hn, qc, gn)
                    elif qc + 1 < nqc:
                        sg_next = st_group(0, qc + 1, 0)
                    if g == 0:
                        po[h] = bpool.tile(
                            [P, QCW], F32, tag="b512", name=f"po_{qc}_{h}"
                        )
                    # filler work BEFORE attn@V: the independent matmul(s)
                    # execute while attn@V waits on the exp semaphore
                    if qc == 0:
                        npop = 2 if idx < 6 else 1
                    elif qc == 1:
                        npop = 2 if idx < 24 else 1
                    else:
                        npop = 1
                    for _ in range(npop):
                        if fl:
                            fl.pop(0)()
                    for j in range(GW):
                        st = g * GW + j
                        nc.tensor.matmul(
                            po[h][0 : DH + 1, :],
                            V[:, st, h, :],
                            pt[:, j, :],
                            start=(st == 0),
                            stop=(st == nst - 1),
                            skip_group_check=True,
                        )
                    if g == ngrp - 1:
                        normalize(po.pop(h), h, qc)

                if not interleave:
                    if qc + 1 < nqc:
                        for m in range(HC):
                            qt_block(qc + 1, m)
                    for slot in range(NH):
                        proj_tile(qc, slot)

            if interleave:
                # epilogue: out-proj of the last q chunk
                for slot in range(NH):
                    proj_tile(nqc - 1, slot)

    nc.compile()
    return nc


def get_nc(seq=SEQ):
    if seq not in _NC_CACHE:
        _NC_CACHE[seq] = build(seq)
    return _NC_CACHE[seq]


def make_in_maps(x, wq, wk, wv, wo):
    bf = ml_dtypes.bfloat16
    in_maps = []
    for c in range(8):
        b, g = c // 2, c % 2
        gs = slice(g * HDIM, (g + 1) * HDIM)
        in_maps.append(
            {
                "xT": np.ascontiguousarray(np.asarray(x)[b].T).astype(bf),
                "wq": np.ascontiguousarray(np.asarray(wq)[:, gs]).astype(bf),
                "wk": np.ascontiguousarray(np.asarray(wk)[:, gs]).astype(bf),
                "wv": np.ascontiguousarray(np.asarray(wv)[:, gs]).astype(bf),
                "wo": np.ascontiguousarray(np.asarray(wo)[gs, :]).astype(bf),
            }
        )
    return in_maps


def combine_outputs(results, bo):
    outs = [np.asarray(results[c]["out"], dtype=np.float32) for c in range(8)]
    y = np.stack([outs[2 * b] + outs[2 * b + 1] for b in range(4)])
    return (y + np.asarray(bo, dtype=np.float32).reshape(1, 1, -1)).astype(np.float32)


def kernel(x, mask, wq, wk, wv, wo, bo):
    nc = get_nc()
    in_maps = make_in_maps(x, wq, wk, wv, wo)
    res = run_bass_kernel_spmd(nc, in_maps, core_ids=list(range(8)))
    return combine_outputs(res.results, bo)
